# revision 2
# baseline (speedup 1.0000x reference)
"""Causal squeeze-excite 1d on 8 TRN2 NeuronCores.

Reference computation (per batch b):
    y = causal_ema(x)                      # y[t] = (1-a) y[t-1] + a x[t], y[0] = x[0]
    h = relu(w1 @ y[:, t] + b1)            # (32,)  per time step
    g = sigmoid(w2 @ h + b2)               # (512,) per time step
    out[:, t] = x[:, t] * g

Sharding: data-parallel over batch. Core i gets x[2i:2i+2]; the tiny MLP
weights are replicated.

Per-core kernel:
  - EMA via DVE tensor_tensor_scan on [128, Tc] tiles (state = d*state + x_t).
    We scan the rescaled sequence u = y/a (u_0 = cw*x_0, u_t = d u_{t-1} + x_t)
    so no a*x pre-scale pass is needed; the 'a' is folded into w1 on the host.
  - MLP1: float32r matmuls contracting C across 4 partition blocks into PSUM.
  - Relu/Sigmoid (+bias) on the scalar engine straight out of PSUM.
  - Gate multiply in-place into the x tile on DVE, then DMA out.
"""

import numpy as np
from contextlib import ExitStack

import concourse.bass as bass
import concourse.bacc as bacc
import concourse.tile as tile
import concourse.mybir as mybir
from concourse.bass_utils import run_bass_kernel_spmd

F32 = mybir.dt.float32
F32R = mybir.dt.float32r

N_CORES = 8
B, C, T = 16, 512, 4096
CSQ = 32          # squeeze dim
P = 128           # SBUF partitions


def build_nc(B_loc, cw, C_=C, T_=T, Tc=2048, TS=512):
    """Build the per-core Bass program. Shapes are compile-time constants."""
    d = 1.0 - 1.0 / cw
    NCB = C_ // P      # channel blocks
    NTH = T_ // Tc     # time chunks
    NTS = Tc // TS     # matmul sub-blocks per chunk

    nc = bacc.Bacc(trn_type="TRN2")
    x = nc.declare_dram_parameter("x", [B_loc, C_, T_], F32, isOutput=False)
    w1sT = nc.declare_dram_parameter("w1sT", [C_, CSQ], F32, isOutput=False)
    b1 = nc.declare_dram_parameter("b1", [CSQ, 1], F32, isOutput=False)
    w2T = nc.declare_dram_parameter("w2T", [CSQ, C_], F32, isOutput=False)
    b2 = nc.declare_dram_parameter("b2", [P, NCB], F32, isOutput=False)
    out = nc.declare_dram_parameter("out", [B_loc, C_, T_], F32, isOutput=True)

    with ExitStack() as ctx:
        tc = ctx.enter_context(tile.TileContext(nc))
        const = ctx.enter_context(tc.tile_pool(name="const", bufs=1))
        xpool = ctx.enter_context(tc.tile_pool(name="xp", bufs=2 * NCB))
        upool = ctx.enter_context(tc.tile_pool(name="up", bufs=2 * NCB))
        hpool = ctx.enter_context(tc.tile_pool(name="hp", bufs=4))
        gpool = ctx.enter_context(tc.tile_pool(name="gp", bufs=4))
        cpool = ctx.enter_context(tc.tile_pool(name="cp", bufs=2))
        php = ctx.enter_context(tc.tile_pool(name="php", bufs=2, space="PSUM"))
        pgp = ctx.enter_context(tc.tile_pool(name="pgp", bufs=4, space="PSUM"))

        dconst = const.tile([P, Tc], F32, tag="dconst")
        nc.vector.memset(dconst[:], d)
        w1_t = []
        for cb in range(NCB):
            wt = const.tile([P, CSQ], F32, tag=f"w1_{cb}")
            nc.sync.dma_start(wt[:], w1sT[cb * P:(cb + 1) * P, :])
            w1_t.append(wt)
        b1_t = const.tile([CSQ, 1], F32, tag="b1")
        nc.sync.dma_start(b1_t[:], b1[:])
        w2_t = const.tile([CSQ, C_], F32, tag="w2")
        nc.sync.dma_start(w2_t[:], w2T[:])
        b2_t = const.tile([P, NCB], F32, tag="b2")
        nc.sync.dma_start(b2_t[:], b2[:])

        for b in range(B_loc):
            u_prev = None
            for th in range(NTH):
                xts, uts = [], []
                for cb in range(NCB):
                    xt = xpool.tile([P, Tc], F32, tag="x")
                    nc.sync.dma_start(
                        xt[:], x[b, cb * P:(cb + 1) * P, th * Tc:(th + 1) * Tc])
                    ut = upool.tile([P, Tc], F32, tag="u")
                    if th == 0:
                        init = cpool.tile([P, 1], F32, tag="c")
                        nc.scalar.mul(init[:], xt[:, 0:1], float(cw))
                        init_ap = init[:]
                    else:
                        init_ap = u_prev[cb][:, Tc - 1:Tc]
                    nc.vector.tensor_tensor_scan(
                        ut[:], dconst[:], xt[:], init_ap,
                        mybir.AluOpType.mult, mybir.AluOpType.add)
                    xts.append(xt)
                    uts.append(ut)
                for ts in range(NTS):
                    ph = php.tile([CSQ, TS], F32, tag="ph")
                    for cb in range(NCB):
                        nc.tensor.matmul(
                            ph[:], w1_t[cb][:],
                            uts[cb][:, ts * TS:(ts + 1) * TS],
                            start=(cb == 0), stop=(cb == NCB - 1))
                    ht = hpool.tile([CSQ, TS], F32, tag="h")
                    nc.scalar.activation(
                        ht[:], ph[:], mybir.ActivationFunctionType.Relu,
                        bias=b1_t[:])
                    for cb in range(NCB):
                        pg = pgp.tile([P, TS], F32, tag="pg")
                        nc.tensor.matmul(
                            pg[:], w2_t[:, cb * P:(cb + 1) * P],
                            ht[:], start=True, stop=True)
                        gt = gpool.tile([P, TS], F32, tag="g")
                        nc.scalar.activation(
                            gt[:], pg[:], mybir.ActivationFunctionType.Sigmoid,
                            bias=b2_t[:, cb:cb + 1])
                        nc.vector.tensor_mul(
                            xts[cb][:, ts * TS:(ts + 1) * TS],
                            xts[cb][:, ts * TS:(ts + 1) * TS], gt[:])
                for cb in range(NCB):
                    nc.scalar.dma_start(
                        out[b, cb * P:(cb + 1) * P, th * Tc:(th + 1) * Tc],
                        xts[cb][:])
                u_prev = uts
    nc.compile()
    return nc


def make_in_maps(x, w1, b1, w2, b2, cw, n_cores=N_CORES):
    """Host-side shard + weight prep. Returns per-core input maps."""
    a = 1.0 / cw
    w1sT = np.ascontiguousarray((w1 * a).T, dtype=np.float32)      # [C, CSQ]
    b1c = np.ascontiguousarray(b1.reshape(-1, 1), dtype=np.float32)
    w2T = np.ascontiguousarray(w2.T, dtype=np.float32)             # [CSQ, C]
    ncb = w2.shape[0] // P
    b2c = np.ascontiguousarray(b2.reshape(ncb, P).T, dtype=np.float32)  # [P, NCB]
    b_loc = x.shape[0] // n_cores
    return [
        {
            "x": np.ascontiguousarray(x[i * b_loc:(i + 1) * b_loc], dtype=np.float32),
            "w1sT": w1sT, "b1": b1c, "w2T": w2T, "b2": b2c,
        }
        for i in range(n_cores)
    ]


_NC_CACHE = {}


def kernel(x, w1, b1, w2, b2, context_window):
    cw = int(context_window)
    x = np.asarray(x)
    key = (cw, x.shape)
    if key not in _NC_CACHE:
        _NC_CACHE[key] = build_nc(x.shape[0] // N_CORES, cw)
    nc = _NC_CACHE[key]
    in_maps = make_in_maps(
        np.asarray(x), np.asarray(w1), np.asarray(b1),
        np.asarray(w2), np.asarray(b2), cw)
    res = run_bass_kernel_spmd(nc, in_maps, core_ids=list(range(N_CORES)))
    return np.concatenate([r["out"] for r in res.results], axis=0)


# revision 4
# speedup vs baseline: 1.0613x; 1.0613x over previous
"""Causal squeeze-excite 1d on 8 TRN2 NeuronCores.

Reference computation (per batch b):
    y = causal_ema(x)                      # y[t] = (1-a) y[t-1] + a x[t], y[0] = x[0]
    h = relu(w1 @ y[:, t] + b1)            # (32,)  per time step
    g = sigmoid(w2 @ h + b2)               # (512,) per time step
    out[:, t] = x[:, t] * g

Sharding: data-parallel over batch. Core i gets x[2i:2i+2]; the tiny MLP
weights are replicated.

Per-core kernel:
  - EMA via DVE tensor_tensor_scan on [128, Tc] tiles (state = d*state + x_t).
    We scan the rescaled sequence u = y/a (u_0 = cw*x_0, u_t = d u_{t-1} + x_t)
    so no a*x pre-scale pass is needed; the 'a' is folded into w1 on the host.
  - MLP1/MLP2: float32r matmuls (1 cyc/row vs fp32's 4) contracting C across
    4 partition blocks into PSUM; producer tiles (u, h, weights) are typed
    float32r so the walrus verifier accepts them as rounded inputs.
  - Relu/Sigmoid (+bias) on the scalar engine straight out of PSUM.
  - Gate multiply in-place into the x tile on DVE, then DMA out.
"""

import numpy as np
from contextlib import ExitStack

import concourse.bass as bass
import concourse.bacc as bacc
import concourse.tile as tile
import concourse.mybir as mybir
from concourse.bass_utils import run_bass_kernel_spmd

F32 = mybir.dt.float32
F32R = mybir.dt.float32r

N_CORES = 8
B, C, T = 16, 512, 4096
CSQ = 32          # squeeze dim
P = 128           # SBUF partitions


def build_nc(B_loc, cw, C_=C, T_=T, Tc=2048, TS=512):
    """Build the per-core Bass program. Shapes are compile-time constants."""
    d = 1.0 - 1.0 / cw
    NCB = C_ // P      # channel blocks
    NTH = T_ // Tc     # time chunks
    NTS = Tc // TS     # matmul sub-blocks per chunk

    nc = bacc.Bacc(trn_type="TRN2")
    x = nc.declare_dram_parameter("x", [B_loc, C_, T_], F32, isOutput=False)
    w1sT = nc.declare_dram_parameter("w1sT", [C_, CSQ], F32R, isOutput=False)
    b1 = nc.declare_dram_parameter("b1", [CSQ, 1], F32, isOutput=False)
    w2T = nc.declare_dram_parameter("w2T", [CSQ, C_], F32R, isOutput=False)
    b2 = nc.declare_dram_parameter("b2", [P, NCB], F32, isOutput=False)
    out = nc.declare_dram_parameter("out", [B_loc, C_, T_], F32, isOutput=True)

    with ExitStack() as ctx:
        tc = ctx.enter_context(tile.TileContext(nc))
        const = ctx.enter_context(tc.tile_pool(name="const", bufs=1))
        xpool = ctx.enter_context(tc.tile_pool(name="xp", bufs=2 * NCB))
        upool = ctx.enter_context(tc.tile_pool(name="up", bufs=2 * NCB))
        hpool = ctx.enter_context(tc.tile_pool(name="hp", bufs=4))
        gpool = ctx.enter_context(tc.tile_pool(name="gp", bufs=4))
        cpool = ctx.enter_context(tc.tile_pool(name="cp", bufs=2))
        php = ctx.enter_context(tc.tile_pool(name="php", bufs=2, space="PSUM"))
        pgp = ctx.enter_context(tc.tile_pool(name="pgp", bufs=4, space="PSUM"))

        dconst = const.tile([P, Tc], F32, tag="dconst")
        nc.vector.memset(dconst[:], d)
        w1_t = []
        for cb in range(NCB):
            wt = const.tile([P, CSQ], F32R, tag=f"w1_{cb}")
            nc.sync.dma_start(wt[:], w1sT[cb * P:(cb + 1) * P, :])
            w1_t.append(wt)
        b1_t = const.tile([CSQ, 1], F32, tag="b1")
        nc.sync.dma_start(b1_t[:], b1[:])
        w2_t = const.tile([CSQ, C_], F32R, tag="w2")
        nc.sync.dma_start(w2_t[:], w2T[:])
        b2_t = const.tile([P, NCB], F32, tag="b2")
        nc.sync.dma_start(b2_t[:], b2[:])

        for b in range(B_loc):
            u_prev = None
            for th in range(NTH):
                xts, uts = [], []
                for cb in range(NCB):
                    xt = xpool.tile([P, Tc], F32, tag="x")
                    nc.sync.dma_start(
                        xt[:], x[b, cb * P:(cb + 1) * P, th * Tc:(th + 1) * Tc])
                    ut = upool.tile([P, Tc], F32R, tag="u")
                    if th == 0:
                        init = cpool.tile([P, 1], F32, tag="c")
                        nc.scalar.mul(init[:], xt[:, 0:1], float(cw))
                        init_ap = init[:]
                    else:
                        init_ap = u_prev[cb][:, Tc - 1:Tc]
                    nc.vector.tensor_tensor_scan(
                        ut[:], dconst[:], xt[:], init_ap,
                        mybir.AluOpType.mult, mybir.AluOpType.add)
                    xts.append(xt)
                    uts.append(ut)
                for ts in range(NTS):
                    ph = php.tile([CSQ, TS], F32, tag="ph")
                    for cb in range(NCB):
                        nc.tensor.matmul(
                            ph[:], w1_t[cb][:],
                            uts[cb][:, ts * TS:(ts + 1) * TS],
                            start=(cb == 0), stop=(cb == NCB - 1))
                    ht = hpool.tile([CSQ, TS], F32R, tag="h")
                    nc.scalar.activation(
                        ht[:], ph[:], mybir.ActivationFunctionType.Relu,
                        bias=b1_t[:])
                    for cb in range(NCB):
                        pg = pgp.tile([P, TS], F32, tag="pg")
                        nc.tensor.matmul(
                            pg[:], w2_t[:, cb * P:(cb + 1) * P],
                            ht[:], start=True, stop=True)
                        gt = gpool.tile([P, TS], F32, tag="g")
                        nc.scalar.activation(
                            gt[:], pg[:], mybir.ActivationFunctionType.Sigmoid,
                            bias=b2_t[:, cb:cb + 1])
                        nc.gpsimd.tensor_mul(
                            xts[cb][:, ts * TS:(ts + 1) * TS],
                            xts[cb][:, ts * TS:(ts + 1) * TS], gt[:])
                for cb in range(NCB):
                    nc.scalar.dma_start(
                        out[b, cb * P:(cb + 1) * P, th * Tc:(th + 1) * Tc],
                        xts[cb][:])
                u_prev = uts
    nc.compile()
    return nc


def make_in_maps(x, w1, b1, w2, b2, cw, n_cores=N_CORES):
    """Host-side shard + weight prep. Returns per-core input maps."""
    a = 1.0 / cw
    w1sT = np.ascontiguousarray((w1 * a).T, dtype=np.float32)      # [C, CSQ]
    b1c = np.ascontiguousarray(b1.reshape(-1, 1), dtype=np.float32)
    w2T = np.ascontiguousarray(w2.T, dtype=np.float32)             # [CSQ, C]
    ncb = w2.shape[0] // P
    b2c = np.ascontiguousarray(b2.reshape(ncb, P).T, dtype=np.float32)  # [P, NCB]
    b_loc = x.shape[0] // n_cores
    return [
        {
            "x": np.ascontiguousarray(x[i * b_loc:(i + 1) * b_loc], dtype=np.float32),
            "w1sT": w1sT, "b1": b1c, "w2T": w2T, "b2": b2c,
        }
        for i in range(n_cores)
    ]


_NC_CACHE = {}


def kernel(x, w1, b1, w2, b2, context_window):
    cw = int(context_window)
    x = np.asarray(x)
    key = (cw, x.shape)
    if key not in _NC_CACHE:
        _NC_CACHE[key] = build_nc(x.shape[0] // N_CORES, cw)
    nc = _NC_CACHE[key]
    in_maps = make_in_maps(
        np.asarray(x), np.asarray(w1), np.asarray(b1),
        np.asarray(w2), np.asarray(b2), cw)
    res = run_bass_kernel_spmd(nc, in_maps, core_ids=list(range(N_CORES)))
    return np.concatenate([r["out"] for r in res.results], axis=0)


# revision 8
# speedup vs baseline: 1.1082x; 1.0442x over previous
"""Causal squeeze-excite 1d on 8 TRN2 NeuronCores.

Reference computation (per batch b):
    y = causal_ema(x)                      # y[t] = (1-a) y[t-1] + a x[t], y[0] = x[0]
    h = relu(w1 @ y[:, t] + b1)            # (32,)  per time step
    g = sigmoid(w2 @ h + b2)               # (512,) per time step
    out[:, t] = x[:, t] * g

Sharding: data-parallel over batch. Core i gets x[2i:2i+2]; the tiny MLP
weights are replicated.

Per-core kernel:
  - EMA via DVE tensor_tensor_scan on [128, Tc] tiles (state = d*state + x_t).
    We scan the rescaled sequence u = y/a (u_0 = cw*x_0, u_t = d u_{t-1} + x_t)
    so no a*x pre-scale pass is needed; the 'a' is folded into w1 on the host.
  - MLP1/MLP2: float32r matmuls (1 cyc/row vs fp32's 4) contracting C across
    4 partition blocks into PSUM; producer tiles (u, h, weights) are typed
    float32r so the walrus verifier accepts them as rounded inputs.
  - Relu/Sigmoid (+bias) on the scalar engine straight out of PSUM.
  - Gate multiply in-place into the x tile on DVE, then DMA out.
"""

import numpy as np
from contextlib import ExitStack

import concourse.bass as bass
import concourse.bacc as bacc
import concourse.tile as tile
import concourse.mybir as mybir
from concourse.bass_utils import run_bass_kernel_spmd

F32 = mybir.dt.float32
F32R = mybir.dt.float32r

N_CORES = 8
B, C, T = 16, 512, 4096
CSQ = 32          # squeeze dim
P = 128           # SBUF partitions


def build_nc(B_loc, cw, C_=C, T_=T, Tc=2048, TS=512):
    """Build the per-core Bass program. Shapes are compile-time constants."""
    d = 1.0 - 1.0 / cw
    NCB = C_ // P      # channel blocks
    NTH = T_ // Tc     # time chunks
    NTS = Tc // TS     # matmul sub-blocks per chunk

    nc = bacc.Bacc(trn_type="TRN2")
    x = nc.declare_dram_parameter("x", [B_loc, C_, T_], F32, isOutput=False)
    w1sT = nc.declare_dram_parameter("w1sT", [C_, CSQ], F32R, isOutput=False)
    b1 = nc.declare_dram_parameter("b1", [CSQ, 1], F32, isOutput=False)
    w2T = nc.declare_dram_parameter("w2T", [CSQ, C_], F32R, isOutput=False)
    b2 = nc.declare_dram_parameter("b2", [P, NCB], F32, isOutput=False)
    out = nc.declare_dram_parameter("out", [B_loc, C_, T_], F32, isOutput=True)

    with ExitStack() as ctx:
        tc = ctx.enter_context(tile.TileContext(nc))
        const = ctx.enter_context(tc.tile_pool(name="const", bufs=1))
        xpool = ctx.enter_context(tc.tile_pool(name="xp", bufs=2 * NCB))
        upool = ctx.enter_context(tc.tile_pool(name="up", bufs=2 * NCB))
        hpool = ctx.enter_context(tc.tile_pool(name="hp", bufs=4))
        gpool = ctx.enter_context(tc.tile_pool(name="gp", bufs=4))
        cpool = ctx.enter_context(tc.tile_pool(name="cp", bufs=2))
        php = ctx.enter_context(tc.tile_pool(name="php", bufs=2, space="PSUM"))
        pgp = ctx.enter_context(tc.tile_pool(name="pgp", bufs=4, space="PSUM"))

        dconst = const.tile([P, Tc], F32, tag="dconst")
        nc.vector.memset(dconst[:], d)
        w1_t = []
        for cb in range(NCB):
            wt = const.tile([P, CSQ], F32R, tag=f"w1_{cb}")
            nc.sync.dma_start(wt[:], w1sT[cb * P:(cb + 1) * P, :])
            w1_t.append(wt)
        b1_t = const.tile([CSQ, 1], F32, tag="b1")
        nc.sync.dma_start(b1_t[:], b1[:])
        w2_t = const.tile([CSQ, C_], F32R, tag="w2")
        nc.sync.dma_start(w2_t[:], w2T[:])
        b2_t = const.tile([P, NCB], F32, tag="b2")
        nc.sync.dma_start(b2_t[:], b2[:])

        for b in range(B_loc):
            u_prev = None
            for th in range(NTH):
                xts, uts = [], []
                for cb in range(NCB):
                    xt = xpool.tile([P, Tc], F32, tag="x")
                    nc.sync.dma_start(
                        xt[:], x[b, cb * P:(cb + 1) * P, th * Tc:(th + 1) * Tc])
                    ut = upool.tile([P, Tc], F32R, tag="u")
                    if th == 0:
                        init = cpool.tile([P, 1], F32, tag="c")
                        nc.scalar.mul(init[:], xt[:, 0:1], float(cw))
                        init_ap = init[:]
                    else:
                        init_ap = u_prev[cb][:, Tc - 1:Tc]
                    # Scans must run on DVE: walrus codegen rejects
                    # tensor_tensor_scan on Pool (GPSIMD).
                    nc.vector.tensor_tensor_scan(
                        ut[:], dconst[:], xt[:], init_ap,
                        mybir.AluOpType.mult, mybir.AluOpType.add)
                    xts.append(xt)
                    uts.append(ut)
                for ts in range(NTS):
                    ph = php.tile([CSQ, TS], F32, tag="ph")
                    for cb in range(NCB):
                        nc.tensor.matmul(
                            ph[:], w1_t[cb][:],
                            uts[cb][:, ts * TS:(ts + 1) * TS],
                            start=(cb == 0), stop=(cb == NCB - 1))
                    ht = hpool.tile([CSQ, TS], F32R, tag="h")
                    nc.scalar.activation(
                        ht[:], ph[:], mybir.ActivationFunctionType.Relu,
                        bias=b1_t[:])
                    for cb in range(NCB):
                        pg = pgp.tile([P, TS], F32, tag="pg")
                        nc.tensor.matmul(
                            pg[:], w2_t[:, cb * P:(cb + 1) * P],
                            ht[:], start=True, stop=True)
                        gt = gpool.tile([P, TS], F32, tag="g")
                        nc.scalar.activation(
                            gt[:], pg[:], mybir.ActivationFunctionType.Sigmoid,
                            bias=b2_t[:, cb:cb + 1])
                        # Gate multiply: DVE does 1 cyc/col but is loaded with
                        # the scans (2 cyc/col); GPSIMD does ~2.7x worse per
                        # col but is otherwise idle. 16 DVE / 48 GPSIMD
                        # balances both near ~85us.
                        mul_eng = nc.vector if (cb + ts) % 4 == 0 else nc.gpsimd
                        mul_eng.tensor_mul(
                            xts[cb][:, ts * TS:(ts + 1) * TS],
                            xts[cb][:, ts * TS:(ts + 1) * TS], gt[:])
                for cb in range(NCB):
                    nc.scalar.dma_start(
                        out[b, cb * P:(cb + 1) * P, th * Tc:(th + 1) * Tc],
                        xts[cb][:])
                u_prev = uts
    nc.compile()
    return nc


def make_in_maps(x, w1, b1, w2, b2, cw, n_cores=N_CORES):
    """Host-side shard + weight prep. Returns per-core input maps."""
    a = 1.0 / cw
    w1sT = np.ascontiguousarray((w1 * a).T, dtype=np.float32)      # [C, CSQ]
    b1c = np.ascontiguousarray(b1.reshape(-1, 1), dtype=np.float32)
    w2T = np.ascontiguousarray(w2.T, dtype=np.float32)             # [CSQ, C]
    ncb = w2.shape[0] // P
    b2c = np.ascontiguousarray(b2.reshape(ncb, P).T, dtype=np.float32)  # [P, NCB]
    b_loc = x.shape[0] // n_cores
    return [
        {
            "x": np.ascontiguousarray(x[i * b_loc:(i + 1) * b_loc], dtype=np.float32),
            "w1sT": w1sT, "b1": b1c, "w2T": w2T, "b2": b2c,
        }
        for i in range(n_cores)
    ]


_NC_CACHE = {}


def kernel(x, w1, b1, w2, b2, context_window):
    cw = int(context_window)
    x = np.asarray(x)
    key = (cw, x.shape)
    if key not in _NC_CACHE:
        _NC_CACHE[key] = build_nc(x.shape[0] // N_CORES, cw)
    nc = _NC_CACHE[key]
    in_maps = make_in_maps(
        np.asarray(x), np.asarray(w1), np.asarray(b1),
        np.asarray(w2), np.asarray(b2), cw)
    res = run_bass_kernel_spmd(nc, in_maps, core_ids=list(range(N_CORES)))
    return np.concatenate([r["out"] for r in res.results], axis=0)


# revision 10
# speedup vs baseline: 1.1207x; 1.0113x over previous
"""Causal squeeze-excite 1d on 8 TRN2 NeuronCores.

Reference computation (per batch b):
    y = causal_ema(x)                      # y[t] = (1-a) y[t-1] + a x[t], y[0] = x[0]
    h = relu(w1 @ y[:, t] + b1)            # (32,)  per time step
    g = sigmoid(w2 @ h + b2)               # (512,) per time step
    out[:, t] = x[:, t] * g

Sharding: data-parallel over batch. Core i gets x[2i:2i+2]; the tiny MLP
weights are replicated.

Per-core kernel:
  - EMA via DVE tensor_tensor_scan on [128, Tc] tiles (state = d*state + x_t).
    We scan the rescaled sequence u = y/a (u_0 = cw*x_0, u_t = d u_{t-1} + x_t)
    so no a*x pre-scale pass is needed; the 'a' is folded into w1 on the host.
  - MLP1/MLP2: float32r matmuls (1 cyc/row vs fp32's 4) contracting C across
    4 partition blocks into PSUM; producer tiles (u, h, weights) are typed
    float32r so the walrus verifier accepts them as rounded inputs.
  - Relu/Sigmoid (+bias) on the scalar engine straight out of PSUM.
  - Gate multiply in-place into the x tile on DVE, then DMA out.
"""

import numpy as np
from contextlib import ExitStack

import concourse.bass as bass
import concourse.bacc as bacc
import concourse.tile as tile
import concourse.mybir as mybir
from concourse.bass_utils import run_bass_kernel_spmd

F32 = mybir.dt.float32
F32R = mybir.dt.float32r

N_CORES = 8
B, C, T = 16, 512, 4096
CSQ = 32          # squeeze dim
P = 128           # SBUF partitions


def build_nc(B_loc, cw, C_=C, T_=T, Tc=2048, TS=512):
    """Build the per-core Bass program. Shapes are compile-time constants."""
    d = 1.0 - 1.0 / cw
    NCB = C_ // P      # channel blocks
    NTH = T_ // Tc     # time chunks
    NTS = Tc // TS     # matmul sub-blocks per chunk

    nc = bacc.Bacc(trn_type="TRN2")
    x = nc.declare_dram_parameter("x", [B_loc, C_, T_], F32, isOutput=False)
    w1sT = nc.declare_dram_parameter("w1sT", [C_, CSQ], F32R, isOutput=False)
    b1 = nc.declare_dram_parameter("b1", [CSQ, 1], F32, isOutput=False)
    w2T = nc.declare_dram_parameter("w2T", [CSQ, C_], F32R, isOutput=False)
    b2 = nc.declare_dram_parameter("b2", [P, NCB], F32, isOutput=False)
    out = nc.declare_dram_parameter("out", [B_loc, C_, T_], F32, isOutput=True)

    with ExitStack() as ctx:
        tc = ctx.enter_context(tile.TileContext(nc))
        const = ctx.enter_context(tc.tile_pool(name="const", bufs=1))
        # One coalesced x tile per (b, th) chunk: [P, NCB*Tc] = 4 MiB, so a
        # single max-efficiency DMA covers the whole chunk. 3 bufs = 3 chunks
        # in flight keeps the DMA rings streaming during compute.
        xpool = ctx.enter_context(tc.tile_pool(name="xp", bufs=3))
        upool = ctx.enter_context(tc.tile_pool(name="up", bufs=6))
        hpool = ctx.enter_context(tc.tile_pool(name="hp", bufs=4))
        gpool = ctx.enter_context(tc.tile_pool(name="gp", bufs=4))
        cpool = ctx.enter_context(tc.tile_pool(name="cp", bufs=2 * NCB))
        php = ctx.enter_context(tc.tile_pool(name="php", bufs=2, space="PSUM"))
        pgp = ctx.enter_context(tc.tile_pool(name="pgp", bufs=4, space="PSUM"))

        dconst = const.tile([P, Tc], F32, tag="dconst")
        nc.vector.memset(dconst[:], d)
        w1_t = []
        for cb in range(NCB):
            wt = const.tile([P, CSQ], F32R, tag=f"w1_{cb}")
            nc.sync.dma_start(wt[:], w1sT[cb * P:(cb + 1) * P, :])
            w1_t.append(wt)
        b1_t = const.tile([CSQ, 1], F32, tag="b1")
        nc.sync.dma_start(b1_t[:], b1[:])
        w2_t = const.tile([CSQ, C_], F32R, tag="w2")
        nc.sync.dma_start(w2_t[:], w2T[:])
        b2_t = const.tile([P, NCB], F32, tag="b2")
        nc.sync.dma_start(b2_t[:], b2[:])

        # DRAM views with channel blocks folded into the free dim:
        # [B, P, NCB, T] so one DMA moves a whole (b, th) chunk.
        xv = x.rearrange("b (cb p) t -> b p cb t", p=P)
        ov = out.rearrange("b (cb p) t -> b p cb t", p=P)

        for b in range(B_loc):
            carry = [None] * NCB
            for th in range(NTH):
                xt = xpool.tile([P, NCB * Tc], F32, tag="x")
                xt3 = xt[:].rearrange("p (cb t) -> p cb t", cb=NCB)
                nc.sync.dma_start(
                    xt3, xv[b, :, :, th * Tc:(th + 1) * Tc])
                uts = []
                for cb in range(NCB):
                    xs = xt[:, cb * Tc:(cb + 1) * Tc]
                    ut = upool.tile([P, Tc], F32R, tag="u")
                    if th == 0:
                        init = cpool.tile([P, 1], F32, tag="c")
                        nc.scalar.mul(init[:], xs[:, 0:1], float(cw))
                        init_ap = init[:]
                    else:
                        init_ap = carry[cb][:]
                    # Scans must run on DVE: walrus codegen rejects
                    # tensor_tensor_scan on Pool (GPSIMD).
                    nc.vector.tensor_tensor_scan(
                        ut[:], dconst[:], xs, init_ap,
                        mybir.AluOpType.mult, mybir.AluOpType.add)
                    if th + 1 < NTH:
                        # Stash the carry so the u tile slot can recycle
                        # without extending its lifetime into the next chunk.
                        cnext = cpool.tile([P, 1], F32, tag="c")
                        nc.scalar.copy(cnext[:], ut[:, Tc - 1:Tc])
                        carry[cb] = cnext
                    uts.append(ut)
                for ts in range(NTS):
                    ph = php.tile([CSQ, TS], F32, tag="ph")
                    for cb in range(NCB):
                        nc.tensor.matmul(
                            ph[:], w1_t[cb][:],
                            uts[cb][:, ts * TS:(ts + 1) * TS],
                            start=(cb == 0), stop=(cb == NCB - 1))
                    ht = hpool.tile([CSQ, TS], F32R, tag="h")
                    nc.scalar.activation(
                        ht[:], ph[:], mybir.ActivationFunctionType.Relu,
                        bias=b1_t[:])
                    for cb in range(NCB):
                        pg = pgp.tile([P, TS], F32, tag="pg")
                        nc.tensor.matmul(
                            pg[:], w2_t[:, cb * P:(cb + 1) * P],
                            ht[:], start=True, stop=True)
                        gt = gpool.tile([P, TS], F32, tag="g")
                        nc.scalar.activation(
                            gt[:], pg[:], mybir.ActivationFunctionType.Sigmoid,
                            bias=b2_t[:, cb:cb + 1])
                        # Gate multiply in place into the x tile. DVE does
                        # 1 cyc/col but is loaded with the scans (2 cyc/col);
                        # GPSIMD is ~2.7x worse per col but otherwise idle.
                        # 16 DVE / 48 GPSIMD balances both near ~90us.
                        sl = slice(cb * Tc + ts * TS, cb * Tc + (ts + 1) * TS)
                        mul_eng = nc.vector if (cb + ts) % 4 == 0 else nc.gpsimd
                        mul_eng.tensor_mul(xt[:, sl], xt[:, sl], gt[:])
                nc.scalar.dma_start(
                    ov[b, :, :, th * Tc:(th + 1) * Tc], xt3)
    nc.compile()
    return nc


def make_in_maps(x, w1, b1, w2, b2, cw, n_cores=N_CORES):
    """Host-side shard + weight prep. Returns per-core input maps."""
    a = 1.0 / cw
    w1sT = np.ascontiguousarray((w1 * a).T, dtype=np.float32)      # [C, CSQ]
    b1c = np.ascontiguousarray(b1.reshape(-1, 1), dtype=np.float32)
    w2T = np.ascontiguousarray(w2.T, dtype=np.float32)             # [CSQ, C]
    ncb = w2.shape[0] // P
    b2c = np.ascontiguousarray(b2.reshape(ncb, P).T, dtype=np.float32)  # [P, NCB]
    b_loc = x.shape[0] // n_cores
    return [
        {
            "x": np.ascontiguousarray(x[i * b_loc:(i + 1) * b_loc], dtype=np.float32),
            "w1sT": w1sT, "b1": b1c, "w2T": w2T, "b2": b2c,
        }
        for i in range(n_cores)
    ]


_NC_CACHE = {}


def kernel(x, w1, b1, w2, b2, context_window):
    cw = int(context_window)
    x = np.asarray(x)
    key = (cw, x.shape)
    if key not in _NC_CACHE:
        _NC_CACHE[key] = build_nc(x.shape[0] // N_CORES, cw)
    nc = _NC_CACHE[key]
    in_maps = make_in_maps(
        np.asarray(x), np.asarray(w1), np.asarray(b1),
        np.asarray(w2), np.asarray(b2), cw)
    res = run_bass_kernel_spmd(nc, in_maps, core_ids=list(range(N_CORES)))
    return np.concatenate([r["out"] for r in res.results], axis=0)
